# revision 29
# baseline (speedup 1.0000x reference)
"""Trainium2 Bass kernel for PixContrastive loss.

Math (per sample n):
  rgb_n, ir_n: [C=64, P=4096] fp32, L2-normalized along C.
  logit = exp((rgb_n^T @ ir_n) / T),  T = 0.1
  pos_n = trace(logit); tot_n = sum(logit)
  loss = mean_n( -log(pos_n / (tot_n + 1e-6)) )

Sharding: data-parallel over N=8 samples across 8 NeuronCores; each core
computes (pos_n, tot_n); the host does the final -log and mean.

Per-core kernel (the scalar engine's 16.7M exps are the bound; everything
else is pipelined into its ramp or tail):
  - inputs DMA'd in interleaved halves so squares start at half-way
  - per-tensor sumsq over channels via ones-vector matmuls -> [128, 32]
    column layout (column m = sumsq[m*128:(m+1)*128] across partitions);
    ir squares on the scalar engine, rgb squares on DVE (parallel chains)
  - inv_norm = rsqrt(sumsq) via exp(-0.5*ln(x)) (both funcs in one ACT
    table set) plus a Newton step on DVE; rgb's inv_norm is fused with
    1/T and used as the exp's per-partition ACT scale (PSUM rows = rgb
    pixels), so rgb itself is only cast to bf16
  - ir is normalized explicitly: PE-transpose inv columns to rows, then
    broadcast each row across 64 partitions with tiny selector-mask
    matmuls (sel_m^T @ invT) into PSUM, DVE multiply (bf16 out), chunked
    so the main loop starts early
  - main loop: 32 row-chunks x 2 halves; 4 bf16 matmuls [64,128]x[64,512]
    per [128,2048] PSUM tile (2 tiles ring = all 8 banks); scalar-engine
    Exp with accum_out collects per-row partial sums into a stats tile
  - diagonal (pos): elementwise rgb16*ir_n product, ones-matmul per chunk
    -> [128,32] allocated from the main PSUM ring so it overlaps the last
    exp tiles, scaled by inv10 columns, Exp+accum
  - final: [128,2] (tot,pos partials) x ones -> [2,1] -> DRAM
"""

import os
import sys

import numpy as np

for _p in ("/opt/trn_rl_repo", "/root/.axon_site/_ro/trn_rl_repo"):
    if os.path.isdir(_p) and _p not in sys.path:
        sys.path.insert(0, _p)

from contextlib import ExitStack

import concourse.bass as bass
import concourse.bacc as bacc
import concourse.tile as tile
from concourse import mybir
from concourse.bass_utils import run_bass_kernel_spmd

C = 64
P = 4096  # 64*64 pixels
N_CORES = 8
TEMP_INV = 10.0  # 1/temperature
LOSS_EPS = 1e-6

F32 = mybir.dt.float32
BF16 = mybir.dt.bfloat16
AF = mybir.ActivationFunctionType
ALU = mybir.AluOpType


def _patch_act_tables():
    """Make natural_log_exp_and_others the only set offering Exp/Ln so the
    table-load pass emits a single ACT_TABLE_LOAD instead of two."""
    import concourse.bacc as _bacc
    import concourse.hw_specs as _hw
    if getattr(_bacc, "_pix_act_patch", False):
        return
    _orig = _bacc.get_activation_tables

    def _patched(arch):
        t = _orig(arch)
        for name, funcs in t.items():
            if name != "natural_log_exp_and_others":
                funcs.discard(AF.Exp)
                funcs.discard(AF.Ln)
                funcs.discard(AF.Square)
        return t

    _bacc.get_activation_tables = _patched
    _bacc._pix_act_patch = True


def _rsqrt_newton(nc, pre_sb, ss, out, extra_scale=None):
    """out = rsqrt(ss) (optionally * extra_scale) for a [128, F] tile/slice.
    seed r0 = exp(-0.5*ln(ss)); one Newton step r0*(1.5 - 0.5*ss*r0^2)."""
    nc_v = nc.vector
    shape = [ss.shape[0], ss.shape[1]]
    lg = pre_sb.tile(shape, F32, tag="lg")
    nc.scalar.activation(lg[:], ss, AF.Ln)
    r0 = pre_sb.tile(shape, F32, tag="r0")
    nc.scalar.activation(r0[:], lg[:], AF.Exp, scale=-0.5)
    t1 = pre_sb.tile(shape, F32, tag="t1")
    nc_v.tensor_mul(t1[:], r0[:], r0[:])
    nc_v.tensor_mul(t1[:], t1[:], ss)
    nc_v.tensor_scalar(t1[:], t1[:], -0.5, 1.5, op0=ALU.mult, op1=ALU.add)
    if extra_scale is None:
        nc_v.tensor_mul(out, t1[:], r0[:])
    else:
        nc_v.scalar_tensor_tensor(out, t1[:], extra_scale, r0[:],
                                  op0=ALU.mult, op1=ALU.mult)


def _build_kernel(nc: bass.Bass, tc: tile.TileContext, ctx: ExitStack,
                  rgb_ap: bass.AP, ir_ap: bass.AP, out_ap: bass.AP) -> None:
    nc_v = nc.vector
    sbuf = ctx.enter_context(tc.tile_pool(name="sbuf", bufs=1))

    ones_b = sbuf.tile([C, 1], BF16, tag="ones_b")
    nc_v.memset(ones_b[:], 1.0)
    ones_f = sbuf.tile([128, 1], F32, tag="ones_f")
    nc_v.memset(ones_f[:], 1.0)

    R = sbuf.tile([C, P], F32, tag="R")
    I = sbuf.tile([C, P], F32, tag="I")
    R16 = sbuf.tile([C, P], BF16, tag="R16")     # raw rgb, bf16
    In16 = sbuf.tile([C, P], BF16, tag="In16")   # normalized ir, bf16
    prod = sbuf.tile([C, P], BF16, tag="prod")   # R16 * In16 (diag path)
    inv10 = sbuf.tile([128, 32], F32, tag="inv10")  # rgb rsqrt * (1/T)
    stats = sbuf.tile([128, 64], F32, tag="stats")
    fin2 = sbuf.tile([128, 2], F32, tag="fin2")     # col 0 tot, col 1 pos
    dsn = sbuf.tile([128, 32], F32, tag="dsn")

    H = P // 2
    # warm the PE HAM clock-gate during the input DMAs: ~4us of dummy
    # matmuls at t=0 flips the PE from 1.2 to 2.4 GHz before the real
    # preamble matmuls issue
    warm = sbuf.tile([C, 128], BF16, tag="warm")
    nc_v.memset(warm[:], 0.0)
    with tc.tile_pool(name="warm_ps", bufs=1, space="PSUM") as warm_ps:
        wp = warm_ps.tile([128, 1], F32, tag="wp")
        for _ in range(40):
            nc.tensor.matmul(wp[:], lhsT=warm[:], rhs=warm[:, 0:1],
                             start=True, stop=True)
    # interleaved half DMAs; ir first (its chain is longest). The two
    # inv-row gather DMAs are emitted mid-stream (between I1 and R1) so
    # their transfers slot in before rgb's second half, whose consumers
    # (exp scales for m>=16, In16 is not involved) run far later.
    nc.sync.dma_start(I[:, 0:H], ir_ap[:, 0:H])
    nc.sync.dma_start(I[:, H:P], ir_ap[:, H:P])
    nc.sync.dma_start(R[:, 0:H], rgb_ap[:, 0:H])
    nc.sync.dma_start(R[:, H:P], rgb_ap[:, H:P])

    with tc.tile_pool(name="pre_ps", bufs=1, space="PSUM") as pre_ps, \
         tc.tile_pool(name="bc_ps", bufs=4, space="PSUM") as bc_pool, \
         tc.tile_pool(name="ps_r1", bufs=1, space="PSUM") as ps_r1_pool, \
         tc.tile_pool(name="pre_sb", bufs=2) as pre_sb:
        from concourse.masks import make_identity
        ident = pre_sb.tile([128, 128], F32, tag="ident")
        make_identity(nc, ident[:])

        sqI = pre_sb.tile([C, P], BF16, tag="sqI")
        sqR = sbuf.tile([C, P], BF16, tag="sqR")
        ss_i = pre_ps.tile([128, 32], F32, tag="ss_i")

        # selector mask: selmask[k, m*64 + c] = (k == m), used to broadcast
        # row m of invT across 64 partitions with one tiny PE matmul
        selmask = sbuf.tile([16, 1024], BF16, tag="selmask")
        nc.gpsimd.memset(selmask[:], 0.0)
        nc.gpsimd.affine_select(
            out=selmask[:].rearrange("p (m c) -> p m c", m=16),
            in_=selmask[:].rearrange("p (m c) -> p m c", m=16),
            compare_op=ALU.not_equal,
            fill=1.0,
            base=0,
            pattern=[[-1, 16], [0, C]],
            channel_multiplier=1,
        )

        # === ir chain, per input half ===
        for h in range(2):
            sl = slice(h * H, (h + 1) * H)
            cols = slice(16 * h, 16 * (h + 1))
            nc.scalar.activation(sqI[:, sl], I[:, sl], AF.Square)
            for m in range(16 * h, 16 * (h + 1)):
                nc.tensor.matmul(ss_i[:, m:m + 1],
                                 lhsT=sqI[:, m * 128:(m + 1) * 128],
                                 rhs=ones_b[:], start=True, stop=True)
            inv_i = pre_sb.tile([128, 16], F32, tag="inv_i")
            _rsqrt_newton(nc, pre_sb, ss_i[:, cols], inv_i)
            invT_ps = pre_ps.tile([16, 128], F32, tag="invT_ps")
            nc.tensor.transpose(invT_ps[:], inv_i[:], ident[:])
            invT = pre_sb.tile([16, 128], BF16, tag="invT")
            nc_v.tensor_copy(invT[:], invT_ps[:])
            # normalize ir: broadcast invT rows across partitions via tiny
            # selector matmuls, 4 chunks batched per PSUM tile + one mul
            for g in range(4):
                bc = bc_pool.tile([C, 512], F32, tag="bc_ps")
                for a in range(4):
                    mk = 4 * g + a
                    nc.tensor.matmul(bc[:, a * 128:(a + 1) * 128],
                                     lhsT=selmask[:, mk * C:(mk + 1) * C],
                                     rhs=invT[:], start=True, stop=True)
                qsl = slice((16 * h + 4 * g) * 128, (16 * h + 4 * g + 4) * 128)
                nc_v.tensor_mul(In16[:, qsl], I[:, qsl], bc[:])

        # === rgb half 0: squares on ACT (DVE is loaded during the ramp) ===
        sqR0 = slice(0, H)
        nc.scalar.activation(sqR[:, sqR0], R[:, sqR0], AF.Square)
        nc_v.tensor_copy(R16[:, sqR0], R[:, sqR0])

        ss_r = pre_ps.tile([128, 16], F32, tag="ss_r")
        for m in range(16):
            nc.tensor.matmul(ss_r[:, m:m + 1],
                             lhsT=sqR[:, m * 128:(m + 1) * 128],
                             rhs=ones_b[:], start=True, stop=True)
        _rsqrt_newton(nc, pre_sb, ss_r, inv10[:, 0:16], extra_scale=TEMP_INV)

        # rgb half 1 feeds exp scales for m>=16 only (~60us of slack):
        # keep it slightly off the critical ramp, in its own 1-bank pool
        with tc.tile_wait_until(0.012):
            sl = slice(H, P)
            nc.scalar.activation(sqR[:, sl], R[:, sl], AF.Square)
            nc_v.tensor_copy(R16[:, sl], R[:, sl])
            ss_r1 = ps_r1_pool.tile([128, 16], F32, tag="ss_r1")
            for m in range(16, 32):
                nc.tensor.matmul(ss_r1[:, m - 16:m - 15],
                                 lhsT=sqR[:, m * 128:(m + 1) * 128],
                                 rhs=ones_b[:], start=True, stop=True)
            # bounce to SBUF so the 1-bank pool releases before the main
            # ring needs all of PSUM (the newton can then run any time)
            ss_r1_sb = sbuf.tile([128, 16], F32, tag="ss_r1_sb")
            nc_v.tensor_copy(ss_r1_sb[:], ss_r1[:])
            _rsqrt_newton(nc, pre_sb, ss_r1_sb, inv10[:, 16:32],
                          extra_scale=TEMP_INV)

    # main loop: 32 row-chunks x (2 halves x 4 matmuls + 1 exp)
    with tc.tile_pool(name="mm_ps", bufs=2, space="PSUM") as mm_ps:
        ds = None
        for m in range(32):
            lhsT = R16[:, m * 128:(m + 1) * 128]
            for h in range(2):
                if m == 31 and h == 1:
                    # allocate the diag tile before the last main tile so its
                    # matmuls overlap the final exp calls
                    ds = mm_ps.tile([128, 32], F32, tag="pt")
                pt = mm_ps.tile([128, 2048], F32, tag="pt")
                for qq in range(4):
                    q = 4 * h + qq
                    nc.tensor.matmul(pt[:, qq * 512:(qq + 1) * 512], lhsT=lhsT,
                                     rhs=In16[:, q * 512:(q + 1) * 512],
                                     start=True, stop=True)
                nc.scalar.activation(pt[:], pt[:], AF.Exp,
                                     scale=inv10[:, m:m + 1],
                                     accum_out=stats[:, 2 * m + h:2 * m + h + 1])

        # diagonal (pos) path: emitted after the main loop, so the scheduler
        # fills idle DVE time with these during the streak
        for j in range(8):
            qsl = slice(j * 512, (j + 1) * 512)
            nc.gpsimd.tensor_mul(prod[:, qsl], R16[:, qsl], In16[:, qsl])
        for m in range(32):
            nc.tensor.matmul(ds[:, m:m + 1], lhsT=prod[:, m * 128:(m + 1) * 128],
                             rhs=ones_b[:], start=True, stop=True)
        nc_v.tensor_mul(dsn[:], ds[:], inv10[:])
        nc.scalar.activation(dsn[:], dsn[:], AF.Exp, accum_out=fin2[:, 1:2])

    # final reduction: [128,2] @ ones -> [2,1] -> DRAM
    nc_v.tensor_reduce(fin2[:, 0:1], stats[:], axis=mybir.AxisListType.X, op=ALU.add)
    with tc.tile_pool(name="fin_ps", bufs=1, space="PSUM") as fin_ps:
        fp = fin_ps.tile([2, 1], F32, tag="fp")
        nc.tensor.matmul(fp[:], lhsT=fin2[:], rhs=ones_f[:], start=True, stop=True)
        fp_sb = sbuf.tile([2, 1], F32, tag="fp_sb")
        nc_v.tensor_copy(fp_sb[:], fp[:])
        nc.sync.dma_start(out_ap[:], fp_sb[:])


def build_nc() -> bass.Bass:
    _patch_act_tables()
    nc = bacc.Bacc("TRN2", target_bir_lowering=False, debug=False,
                   num_devices=N_CORES)
    rgb = nc.dram_tensor("rgb", [C, P], F32, kind="ExternalInput").ap()
    ir = nc.dram_tensor("ir", [C, P], F32, kind="ExternalInput").ap()
    out = nc.dram_tensor("out", [2, 1], F32, kind="ExternalOutput").ap()
    with tile.TileContext(nc) as tc:
        with ExitStack() as ctx:
            _build_kernel(nc, tc, ctx, rgb, ir, out)
    nc.compile()
    return nc


_NC = None


def _get_nc() -> bass.Bass:
    global _NC
    if _NC is None:
        _NC = build_nc()
    return _NC


def run_cores(rgb: np.ndarray, ir: np.ndarray, **spmd_kwargs):
    """rgb/ir: [8, 64, 4096] fp32. Returns (pos[8], tot[8], BassKernelResults)."""
    nc = _get_nc()
    in_maps = [{"rgb": np.ascontiguousarray(rgb[n]),
                "ir": np.ascontiguousarray(ir[n])} for n in range(N_CORES)]
    r = run_bass_kernel_spmd(nc, in_maps, list(range(N_CORES)), **spmd_kwargs)
    pos = np.array([r.results[n]["out"][1, 0] for n in range(N_CORES)], np.float64)
    tot = np.array([r.results[n]["out"][0, 0] for n in range(N_CORES)], np.float64)
    return pos, tot, r


def kernel(rgb_map: np.ndarray, ir_map: np.ndarray, targets=None, **_unused) -> np.ndarray:
    rgb = np.asarray(rgb_map, np.float32).reshape(N_CORES, C, P)
    ir = np.asarray(ir_map, np.float32).reshape(N_CORES, C, P)
    pos, tot, _ = run_cores(rgb, ir)
    loss = float(np.mean(-np.log(pos / (tot + LOSS_EPS))))
    return np.asarray(loss, dtype=np.float32)
